# revision 1
# baseline (speedup 1.0000x reference)
"""Trainium2 Bass kernel for the CaptionDecoderCell problem.

Strategy: data-parallel over batch B=256 across 8 cores (B_local=32 each).
Per core:
  - teacher-forced input-side gate preactivations (Xg) for all T=20 steps are
    computed in one batched GEMM up front (they don't depend on the recurrence)
  - the sequential part is only h @ W_hh.T per step (fp16 weights stationary,
    gate-dim on partitions) plus LSTM elementwise on ACT/DVE; the precomputed
    Xg is injected into PSUM via an identity matmul so ACT reads gate
    preactivations straight out of PSUM (no DVE add on the critical path)
  - the V=10000 output projection runs once over all B*T=640 hidden states,
    with V on the matmul free axis so logits DMA out contiguously
All GEMMs in fp16 (fp32 PSUM accumulate): ~2e-3 rel err on hid, ~6e-4 on
logits vs the fp32 reference (measured in numpy), at 4x the fp32 PE rate.
"""

import numpy as np

B, E, M2, T, V = 256, 512, 512, 20, 10000
NCORES = 8
BL = B // NCORES          # 32 batch per core
P = 128
KE = E // P               # 4 k-chunks over E
KM = M2 // P              # 4 k-chunks over M2
G = 4 * M2 // P           # 16 gate-dim chunks
N = T * BL                # 640 = batch*time columns
VT = 500                  # logits v-slice (fits one PSUM bank: 500 f32)
NV = V // VT              # 20 v-slices
BT_TILES = N // P         # 5 tiles of 128 (s,b) columns

# gate reorder: reference packs rows as [i, f, g, og]; we use [i, f, og, g]
# so chunks 0:4=i, 4:8=f, 8:12=og, 12:16=g.
_PERM = np.r_[0:M2, M2:2 * M2, 3 * M2:4 * M2, 2 * M2:3 * M2]

_CACHE = {}


def _build_program():
    from concourse import bacc
    import concourse.tile as tile
    from concourse import mybir
    from concourse.masks import make_identity

    f16 = mybir.dt.float16
    f32 = mybir.dt.float32
    AF = mybir.ActivationFunctionType

    nc = bacc.Bacc("TRN2", target_bir_lowering=False, debug=False,
                   num_devices=NCORES)

    # ---- DRAM I/O ----
    d_oT = nc.dram_tensor("oT", [E, BL], f16, kind="ExternalInput")
    d_xembT = nc.dram_tensor("xembT", [E, N], f16, kind="ExternalInput")
    d_whT = nc.dram_tensor("whT", [E, M2], f16, kind="ExternalInput")
    d_wcT = nc.dram_tensor("wcT", [E, M2], f16, kind="ExternalInput")
    d_whhT = nc.dram_tensor("whhT", [M2, 4 * M2], f16, kind="ExternalInput")
    d_wixT = nc.dram_tensor("wixT", [E, 4 * M2], f16, kind="ExternalInput")
    d_wioT = nc.dram_tensor("wioT", [E, 4 * M2], f16, kind="ExternalInput")
    d_wlT = nc.dram_tensor("wlT", [M2, V], f16, kind="ExternalInput")
    d_bsum = nc.dram_tensor("bsum", [P, G], f32, kind="ExternalInput")
    d_bhT = nc.dram_tensor("bhT", [P, KM], f32, kind="ExternalInput")
    d_bcT = nc.dram_tensor("bcT", [P, KM], f32, kind="ExternalInput")

    d_logits = nc.dram_tensor("logits_nv", [N, V], f32, kind="ExternalOutput")
    # [s, p, k*BL+b] so each partition's step-write is one contiguous run
    d_hstates = nc.dram_tensor("hstates", [T, P, KM * BL], f32,
                               kind="ExternalOutput")

    def pk(ap, k=P):  # (K, F) dram -> (p, ko, F)
        return ap.rearrange("(ko p) f -> p ko f", p=k)

    # gate groups in emission order: name -> (chunk_base, n_chunks)
    # chunks 0:4=i, 4:8=f, 8:12=og share one sigmoid; 12:16=g gets tanh
    GRP = {"ifo": (0, 12), "g": (12, 4)}

    with tile.TileContext(nc) as tc:
        with tc.tile_pool(name="persist", bufs=1) as persist, \
             tc.tile_pool(name="work", bufs=3) as work, \
             tc.tile_pool(name="cpool", bufs=2) as cpool, \
             tc.tile_pool(name="lout", bufs=4) as lout:

            # ---- persistent SBUF ----
            bsum_sb = persist.tile([P, G], f32)
            bhT_sb = persist.tile([P, KM], f32)
            bcT_sb = persist.tile([P, KM], f32)
            ident = persist.tile([P, P], f16)
            make_identity(nc, ident)
            # force the Sigmoid/Tanh ACT tables to load now, during setup
            # DMAs, instead of lazily on the step-0 critical path
            warm = persist.tile([P, 1], f32)
            nc.scalar.activation(warm, ident[:, 0:1], AF.Sigmoid)
            nc.scalar.activation(warm, ident[:, 0:1], AF.Tanh)

            XgOb = persist.tile([P, G, T, BL], f16)   # input-side gates, all steps
            # h_s for all steps (fp16), one tile per logits bt-group of 4
            # steps so interleaved logits reads don't create false deps
            Hbt = [persist.tile([P, KM, 4 * BL], f16, name=f"hbt{j}")
                   for j in range(BT_TILES)]
            wlT_sb = persist.tile([P, KM, V], f16)    # resident Wl^T
            whhT_sb = persist.tile([P, KM, 4 * M2], f16)
            h0_16 = persist.tile([P, KM, BL], f16)    # initial hidden state

            # ---- setup phase (scoped pool; freed before logits) ----
            with tc.tile_pool(name="setup", bufs=1) as setup, \
                 tc.tile_pool(name="wstream", bufs=8) as wstream, \
                 tc.tile_pool(name="pss", bufs=1, space="PSUM") as pss, \
                 tc.tile_pool(name="psx", bufs=4, space="PSUM") as psx:

                # dummy matmul burst on a zeroed tile: keeps the PE busy while
                # the first weight DMAs land, so HAM is un-throttled (2.4 GHz)
                # by the time real GEMMs start
                wu = setup.tile([P, 512], f16)
                nc.vector.memset(wu, 0.0)
                ps_wu = psx.tile([P, 512], f32, tag="ps_xg")
                for r in range(24):
                    nc.tensor.matmul(ps_wu, wu[:, 0:128], wu,
                                     start=(r == 0), stop=(r == 23),
                                     skip_group_check=True)

                oT_sb = setup.tile([P, KE, BL], f16)
                nc.sync.dma_start(out=oT_sb, in_=pk(d_oT.ap()))
                whT_sb = setup.tile([P, KE, M2], f16)
                nc.sync.dma_start(out=whT_sb, in_=pk(d_whT.ap()))
                wcT_sb = setup.tile([P, KE, M2], f16)
                nc.sync.dma_start(out=wcT_sb, in_=pk(d_wcT.ap()))
                wio_sb = setup.tile([P, KE, 4 * M2], f16)
                for ch in range(4):
                    c0_, c1 = ch * M2, (ch + 1) * M2
                    nc.sync.dma_start(out=wio_sb[:, :, c0_:c1],
                                      in_=pk(d_wioT.ap())[:, :, c0_:c1])
                xembT_sb = setup.tile([P, KE, N], f16)
                nc.sync.dma_start(out=xembT_sb, in_=pk(d_xembT.ap()))
                # small strided bias DMAs (128 descriptors each) go after the
                # big contiguous loads so they don't delay the first matmul
                nc.sync.dma_start(out=bsum_sb, in_=d_bsum.ap())
                nc.sync.dma_start(out=bhT_sb, in_=d_bhT.ap())
                nc.sync.dma_start(out=bcT_sb, in_=d_bcT.ap())

                # h0 = o @ Wh.T + bh ; c0 = o @ Wc.T + bc  (one dense burst)
                ps_init = pss.tile([P, 2, KM, BL], f32, tag="ps_init")
                for wi, wsb in enumerate((whT_sb, wcT_sb)):
                    for m in range(KM):
                        for k in range(KE):
                            nc.tensor.matmul(ps_init[:, wi, m, :],
                                             wsb[:, k, m * P:(m + 1) * P],
                                             oT_sb[:, k, :],
                                             start=(k == 0), stop=(k == KE - 1))
                c0_sb = cpool.tile([P, KM, BL], f32, tag="c")
                for m in range(KM):
                    nc.scalar.activation(h0_16[:, m, :], ps_init[:, 0, m, :],
                                         AF.Identity, bias=bhT_sb[:, m:m + 1])
                    nc.scalar.activation(c0_sb[:, m, :], ps_init[:, 1, m, :],
                                         AF.Identity, bias=bcT_sb[:, m:m + 1])

                # Og_b = o @ W_ih[:, E:].T + (b_ih + b_hh)  (dense burst)
                ogb_sb = setup.tile([P, G, BL], f32)
                ps_og = pss.tile([P, G, BL], f32, tag="ps_og")
                for g in range(G):
                    for k in range(KE):
                        nc.tensor.matmul(ps_og[:, g, :],
                                         wio_sb[:, k, g * P:(g + 1) * P],
                                         oT_sb[:, k, :],
                                         start=(k == 0), stop=(k == KE - 1))
                for g in range(G):
                    nc.scalar.activation(ogb_sb[:, g, :], ps_og[:, g, :],
                                         AF.Identity, bias=bsum_sb[:, g:g + 1])

                # Xg[:, g, s, b] = xemb @ W_ih[:, :E].T ;  XgOb = Xg + Og_b
                for g in range(G):
                    wix_g = wstream.tile([P, KE, P], f16, tag="wix")
                    nc.sync.dma_start(out=wix_g,
                                      in_=pk(d_wixT.ap())[:, :, g * P:(g + 1) * P])
                    for (s0, ns) in ((0, 16), (16, 4)):   # n-chunks of 512, 128
                        ps_xg = psx.tile([P, 512], f32, tag="ps_xg")
                        pxg = ps_xg[:, :ns * BL]
                        for k in range(KE):
                            nc.tensor.matmul(
                                pxg, wix_g[:, k, :],
                                xembT_sb[:, k, s0 * BL:(s0 + ns) * BL],
                                start=(k == 0), stop=(k == KE - 1))
                        nc.vector.tensor_add(
                            XgOb[:, g, s0:s0 + ns, :],
                            pxg.rearrange("p (s b) -> p s b", b=BL),
                            ogb_sb[:, g, None, :].to_broadcast([P, ns, BL]))

            # whhT needed from rec start; wlT only at logits: stream both in
            # the background (DMA has spare bandwidth during setup/rec)
            nc.sync.dma_start(out=whhT_sb, in_=pk(d_whhT.ap()))
            for ch in range(8):
                v0, v1 = ch * (V // 8), (ch + 1) * (V // 8)
                nc.sync.dma_start(out=wlT_sb[:, :, v0:v1],
                                  in_=pk(d_wlT.ap())[:, :, v0:v1])

            # ---- logits tile emitter (interleaved into rec gaps + tail) ----
            logits_sched = [(v, bt) for bt in range(BT_TILES)
                            for v in range(NV)]
            logits_pos = [0]

            def emit_logits_tiles(psl, count, bt_limit):
                while logits_pos[0] < len(logits_sched) and count > 0:
                    v, bt = logits_sched[logits_pos[0]]
                    if bt >= bt_limit:
                        return
                    logits_pos[0] += 1
                    count -= 1
                    ps_l = psl.tile([P, VT], f32, tag="ps_l")
                    for k in range(KM):
                        nc.tensor.matmul(
                            ps_l,
                            Hbt[bt][:, k, :],
                            wlT_sb[:, k, v * VT:(v + 1) * VT],
                            start=(k == 0), stop=(k == KM - 1))
                    o_l = lout.tile([P, VT], f32, tag="o_l")
                    if v % 2 == 0:
                        nc.vector.tensor_copy(o_l, ps_l)
                    else:
                        nc.scalar.copy(o_l, ps_l)
                    nc.sync.dma_start(
                        out=d_logits.ap()[bt * P:(bt + 1) * P,
                                          v * VT:(v + 1) * VT],
                        in_=o_l)

            # ---- recurrence (logits tiles fill the PE gap while ACT/DVE
            # run each step's elementwise chain) ----
            with tc.tile_pool(name="psr", bufs=1, space="PSUM") as psr, \
                 tc.tile_pool(name="psl", bufs=6, space="PSUM") as psl:
                c_prev = c0_sb
                h_prev = h0_16
                for s in range(T):
                    ps = {}
                    for name, (b0, nch) in GRP.items():
                        pt = psr.tile([P, nch, BL], f32, tag=f"ps_{name}")
                        # inject precomputed input-side gates, then accumulate
                        # W_hh @ h on top
                        nc.tensor.matmul(pt, ident,
                                         XgOb[:, b0:b0 + nch, s, :],
                                         start=True, stop=False)
                        for j in range(nch):
                            gch = b0 + j
                            for k in range(KM):
                                nc.tensor.matmul(
                                    pt[:, j, :],
                                    whhT_sb[:, k, gch * P:(gch + 1) * P],
                                    h_prev[:, k, :],
                                    start=False, stop=(k == KM - 1),
                                    skip_group_check=True)
                        ps[name] = pt
                    # logits tiles for already-complete bt groups run on PE
                    # while ACT/DVE process this step's gates
                    emit_logits_tiles(psl, 2, s // 4)
                    sig_ifo = work.tile([P, 12, BL], f32, tag="sig_ifo")
                    nc.scalar.activation(sig_ifo, ps["ifo"], AF.Sigmoid)
                    tanh_g = work.tile([P, KM, BL], f32, tag="tanh_g")
                    nc.scalar.activation(tanh_g, ps["g"], AF.Tanh)
                    fc = work.tile([P, KM, BL], f32, tag="fc")
                    nc.vector.tensor_mul(fc, sig_ifo[:, 4:8, :], c_prev)  # f*c
                    ig = work.tile([P, KM, BL], f32, tag="ig")
                    nc.vector.tensor_mul(ig, sig_ifo[:, 0:4, :], tanh_g)  # i*g
                    c_new = cpool.tile([P, KM, BL], f32, tag="c")
                    nc.vector.tensor_add(c_new, fc, ig)
                    tanh_c = work.tile([P, KM, BL], f32, tag="tanh_c")
                    nc.scalar.activation(tanh_c, c_new, AF.Tanh)
                    sig_o = sig_ifo[:, 8:12, :]
                    # fp16 h for the next step + logits (critical path)
                    hslot = Hbt[s // 4][:, :, (s % 4) * BL:(s % 4 + 1) * BL]
                    nc.vector.tensor_mul(hslot, sig_o, tanh_c)
                    # fp32 h for the hid output (off critical path, GpSimd)
                    h_f32 = work.tile([P, KM, BL], f32, tag="hf32")
                    nc.gpsimd.tensor_mul(h_f32, sig_o, tanh_c)
                    nc.sync.dma_start(out=d_hstates.ap()[s], in_=h_f32)
                    h_prev = hslot
                    c_prev = c_new

                # ---- logits tail ----
                emit_logits_tiles(psl, 10 ** 9, BT_TILES)

    nc.compile()
    return nc


def _prep_shared(inputs):
    """Host-side weight layout prep (shared across cores), all fp16."""
    f16 = np.float16
    W_ih = np.asarray(inputs["W_ih"], np.float32)[_PERM]
    W_hh = np.asarray(inputs["W_hh"], np.float32)[_PERM]
    bsum = (np.asarray(inputs["b_ih"], np.float32)
            + np.asarray(inputs["b_hh"], np.float32))[_PERM]
    sh = {
        "whT": np.ascontiguousarray(np.asarray(inputs["Wh"]).T).astype(f16),
        "wcT": np.ascontiguousarray(np.asarray(inputs["Wc"]).T).astype(f16),
        "whhT": np.ascontiguousarray(W_hh.T).astype(f16),
        "wixT": np.ascontiguousarray(W_ih[:, :E].T).astype(f16),
        "wioT": np.ascontiguousarray(W_ih[:, E:].T).astype(f16),
        "wlT": np.ascontiguousarray(np.asarray(inputs["Wl"]).T).astype(f16),
        "bsum": np.ascontiguousarray(bsum.reshape(G, P).T).astype(np.float32),
        "bhT": np.ascontiguousarray(
            np.asarray(inputs["bh"], np.float32).reshape(KM, P).T),
        "bcT": np.ascontiguousarray(
            np.asarray(inputs["bc"], np.float32).reshape(KM, P).T),
    }
    return sh


def kernel(**inputs):
    from concourse.bass_utils import run_bass_kernel_spmd

    if "nc" not in _CACHE:
        _CACHE["nc"] = _build_program()
    nc = _CACHE["nc"]

    o = np.asarray(inputs["o"], np.float32)
    t = np.asarray(inputs["t"])
    length = np.asarray(inputs["length"])
    emb = np.asarray(inputs["emb"], np.float32)
    bl = np.asarray(inputs["bl"], np.float32)
    sh = _prep_shared(inputs)

    in_maps = []
    for c in range(NCORES):
        bsl = slice(c * BL, (c + 1) * BL)
        o_loc = o[bsl]                                   # (BL, E)
        t_loc = t[bsl]                                   # (BL, T)
        # teacher-forced inputs: step 0 uses o, steps 1..T-1 use emb[t[:, s-1]]
        xemb = np.empty((T, BL, E), np.float32)
        xemb[0] = o_loc
        xemb[1:] = emb[t_loc[:, :T - 1]].transpose(1, 0, 2)
        m = dict(sh)
        m["oT"] = np.ascontiguousarray(o_loc.T).astype(np.float16)
        m["xembT"] = np.ascontiguousarray(
            xemb.reshape(N, E).T).astype(np.float16)
        in_maps.append(m)

    res = run_bass_kernel_spmd(nc, in_maps, core_ids=list(range(NCORES)),
                               **_CACHE.get("run_kwargs", {}))
    _CACHE["last_results"] = res

    idx = np.clip(length - 1, 0, None).astype(np.int64)
    logits = np.empty((B, V, T), np.float32)
    hid = np.empty((B, M2), np.float32)
    for c in range(NCORES):
        bsl = slice(c * BL, (c + 1) * BL)
        lg = res.results[c]["logits_nv"].reshape(T, BL, V)
        logits[bsl] = lg.transpose(1, 2, 0)
        hs = res.results[c]["hstates"].reshape(T, P, KM, BL)
        idx_loc = idx[bsl]
        # hid[b, k*128+p] = hs[idx_b, p, k, b]
        sel = hs[idx_loc, :, :, np.arange(BL)]           # (BL, P, KM)
        hid[bsl] = sel.transpose(0, 2, 1).reshape(BL, M2)
    logits += bl[None, :, None]
    return logits, hid
